# revision 1
# baseline (speedup 1.0000x reference)
"""Trainium2 Bass kernel for nn_DiffLogicPBF (difflogic network).

Algorithm
---------
The network input is binarized to 2 bits, so every batch row's entire
activation trajectory takes one of only 4 values ("patterns").  We evaluate
the network on the 4 patterns instead of 8192 rows, then blend per-row.

The per-layer gathers (connection indices) are known when the kernel is
built, so they are composed on the host into a stream tree: layer l needs
its layer-(l-1) inputs in 2 permuted orders, giving 2^(5-l) "streams" per
layer (63 total), each a gather-free elementwise evaluation.  Weights are
uploaded pre-permuted per stream; softmax/logic-coefficient math runs on
device via exp + strided corner-mask reductions (the 16 soft logic
functions' truth tables at the 4 binary corners are exact bit masks).

Sharding: neurons (K=4096) are split across the 8 cores (512 each).  Each
core computes its partial GroupSum table [4 patterns x 2 classes], blends
the full batch against it ([B,2] partial logits), and the host sums the 8
partial outputs (the blend is linear in the table).

Engine split: DVE does the corner reductions and the multilinear eval;
GpSimd does the coefficient algebra and the a*b products of the two big
layers; ACT does exp; PE broadcasts the table.  The weight blobs arrive in
4 contiguous chunks so exp/reduces pipeline with the DMA.
"""

from contextlib import ExitStack

import ml_dtypes
import numpy as np

import concourse.bacc as bacc
import concourse.bass as bass
import concourse.mybir as mybir
import concourse.tile as tile
from concourse.bass_utils import run_bass_kernel_spmd

F32 = mybir.dt.float32
ADD = mybir.AluOpType.add
SUB = mybir.AluOpType.subtract
MUL = mybir.AluOpType.mult
X = mybir.AxisListType.X
XY = mybir.AxisListType.XY

N_CORES = 8
B, K, L = 8192, 4096, 6
NS = [32, 16, 8, 4, 2, 1]          # streams per layer
NSTOT = sum(NS)                    # 63
KLOC = K // N_CORES                # 512 neurons per core
J = KLOC // 128                    # 4 free chunks per partition
FO = np.cumsum([0] + NS).tolist()  # stream offsets by layer
NSJ = NSTOT * J                    # 252
BROW = B // 128                    # 64 batch rows per partition

# weight pipeline groups as (start_stream, n_streams); layer 0 is split in
# half so the first exp/reduce chunk starts after ~0.5 MB of DMA
WG = [(0, 32), (32, 16), (48, 15)]

_compiled = None


def _build_program():
    nc = bacc.Bacc("TRN2", target_bir_lowering=False, debug=False,
                   num_devices=N_CORES)
    BF16 = mybir.dt.bfloat16
    walls = [nc.dram_tensor(f"wall{gi}", [128, n * J * 16], BF16,
                            kind="ExternalInput").ap()
             for gi, (s0_, n) in enumerate(WG)]
    a0in = nc.dram_tensor("a0in", [128, NS[0] * J * 4], mybir.dt.bfloat16, kind="ExternalInput").ap()
    b0in = nc.dram_tensor("b0in", [128, NS[0] * J * 4], mybir.dt.bfloat16, kind="ExternalInput").ap()
    xin = nc.dram_tensor("xin", [128, BROW, 2], F32, kind="ExternalInput").ap()
    clsg = nc.dram_tensor("clsg", [128, 2], F32, kind="ExternalInput").ap()
    out = nc.dram_tensor("out", [B, 2], F32, kind="ExternalOutput").ap()

    EXP = mybir.ActivationFunctionType.Exp
    GT = mybir.AluOpType.is_gt

    with tile.TileContext(nc) as tc:
        with ExitStack() as ctx:
            p = ctx.enter_context(tc.tile_pool(name="p", bufs=1))
            psp = ctx.enter_context(tc.tile_pool(name="ps", bufs=1, space="PSUM"))

            # ---- input DMAs (split across trigger engines / queues) ----
            dma_engines = [nc.sync, nc.scalar, nc.sync, nc.scalar]
            xt = p.tile([128, BROW, 2], F32)
            nc.scalar.dma_start(xt[:], xin[:])
            ct = p.tile([128, 2], F32)
            nc.scalar.dma_start(ct[:], clsg[:])
            wts = []
            for gi, (s0_, n) in enumerate(WG):
                wt = p.tile([128, n * J * 16], BF16, tag=f"wt{gi}")
                nc.sync.dma_start(wt[:], walls[gi][:])
                wts.append(wt)
                if gi == 0:
                    a0t = p.tile([128, NS[0] * J * 4], BF16)
                    nc.scalar.dma_start(a0t[:], a0in[:])
                    b0t = p.tile([128, NS[0] * J * 4], BF16)
                    nc.scalar.dma_start(b0t[:], b0in[:])

            # blend prep + constants (fill early DVE idle time)
            s0 = p.tile([128, BROW], F32)
            nc.vector.tensor_scalar(s0[:], xt[:, :, 0], 0.0, None, op0=GT)
            s1 = p.tile([128, BROW], F32)
            nc.vector.tensor_scalar(s1[:], xt[:, :, 1], 0.0, None, op0=GT)
            t01 = p.tile([128, BROW], F32)
            nc.vector.tensor_tensor(t01[:], s0[:], s1[:], op=MUL)
            ones_m = p.tile([128, 128], F32)
            nc.vector.memset(ones_m[:], 1.0)
            # warm the PE early: 1x1 matmul of ones written back into ones_m
            # (semantically a no-op, keeps the chain live through the real
            # matmul below)
            wm = psp.tile([1, 1], F32)
            nc.tensor.matmul(wm[:], ones_m[0:1, 0:1], ones_m[0:1, 0:1],
                             start=True, stop=True)
            nc.scalar.copy(ones_m[0:1, 0:1], wm[:])

            # ---- per-group: exp -> corner masks (DVE) -> coeffs (GpSimd) ----
            Cw = []                      # per-wgroup (C0..C3) tiles
            Dw, rw, Tw = [], [], []
            for gi, (s0_, n) in enumerate(WG):
                nsjg = n * J
                E = p.tile([128, nsjg * 16], F32, tag=f"E{gi}")
                nc.scalar.activation(E[:], wts[gi][:], EXP)
                Ev = E[:].rearrange("p (n i) -> p n i", i=16)
                e0 = Ev[:, :, 0:1]

                rd = nc.vector.tensor_reduce
                gt = nc.gpsimd.tensor_tensor
                V11 = p.tile([128, nsjg], F32, tag=f"V11{gi}")
                rd(V11[:], Ev[:, :, 1::2], axis=X, op=ADD)
                Sev = p.tile([128, nsjg], F32, tag=f"Sev{gi}")
                rd(Sev[:], Ev[:, :, 0::2], axis=X, op=ADD)
                D = p.tile([128, nsjg], F32, tag=f"D{gi}")
                nc.vector.tensor_tensor(D[:], V11[:], Sev[:], op=ADD)
                Dw.append(D)
                r = p.tile([128, nsjg], F32, tag=f"r{gi}")
                rw.append(r)
                V10 = p.tile([128, nsjg], F32, tag=f"V10{gi}")
                m10 = bass.AP(tensor=e0.tensor, offset=e0.offset + 2,
                              ap=[e0.ap[0], [16, nsjg], [4, 4], [1, 2]])
                rd(V10[:], m10, axis=XY, op=ADD)
                V01 = p.tile([128, nsjg], F32, tag=f"V01{gi}")
                m01 = bass.AP(tensor=e0.tensor, offset=e0.offset + 4,
                              ap=[e0.ap[0], [16, nsjg], [8, 2], [1, 4]])
                rd(V01[:], m01, axis=XY, op=ADD)
                V00 = p.tile([128, nsjg], F32, tag=f"V00{gi}")
                rd(V00[:], Ev[:, :, 8:16], axis=X, op=ADD)

                t1 = p.tile([128, nsjg], F32, tag=f"t1{gi}")
                gt(t1[:], V11[:], V10[:], op=SUB)
                t2 = p.tile([128, nsjg], F32, tag=f"t2{gi}")
                gt(t2[:], V01[:], V00[:], op=SUB)
                t3 = p.tile([128, nsjg], F32, tag=f"t3{gi}")
                gt(t3[:], V10[:], V00[:], op=SUB)
                c3u = p.tile([128, nsjg], F32, tag=f"c3u{gi}")
                gt(c3u[:], t1[:], t2[:], op=SUB)
                Tw.append((V00, t3, t2, c3u))

            # 1/D on ACT via exp(-ln(D))
            lns = []
            for gi, (s0_, n) in enumerate(WG):
                nsjg = n * J
                lnD = p.tile([128, nsjg], F32, tag=f"lnD{gi}")
                nc.scalar.activation(lnD[:], Dw[gi][:],
                                     mybir.ActivationFunctionType.Ln)
                lns.append(lnD)
            for gi, (s0_, n) in enumerate(WG):
                nc.scalar.activation(rw[gi][:], lns[gi][:], EXP, scale=-1.0)

            for gi, (s0_, n) in enumerate(WG):
                nsjg = n * J
                gt = nc.gpsimd.tensor_tensor
                r = rw[gi]
                V00g, t3, t2, c3u = Tw[gi]
                C0 = p.tile([128, nsjg], BF16, tag=f"C0{gi}")
                gt(C0[:], V00g[:], r[:], op=MUL)
                C1 = p.tile([128, nsjg], BF16, tag=f"C1{gi}")
                gt(C1[:], t3[:], r[:], op=MUL)
                C2 = p.tile([128, nsjg], BF16, tag=f"C2{gi}")
                gt(C2[:], t2[:], r[:], op=MUL)
                C3 = p.tile([128, nsjg], BF16, tag=f"C3{gi}")
                gt(C3[:], c3u[:], r[:], op=MUL)
                Cw.append((C0, C1, C2, C3))

            def c_slices(l):
                """pieces (n_streams, [c0..c3 APs]) covering layer l's
                streams in order; layer 0 may span several wgroups."""
                res = []
                lo, hi = FO[l], FO[l] + NS[l]
                for gi, (gs, gn) in enumerate(WG):
                    a, b = max(lo, gs), min(hi, gs + gn)
                    if a < b:
                        res.append((b - a, [t[:, (a - gs) * J:(b - gs) * J]
                                            for t in Cw[gi]]))
                return res

            # ---- evaluate the stream tree on the 4 patterns ----
            def eval_piece(l, pi, A, Bv, cs, Hv, on_gpsimd=False):
                nf = A.shape[1]
                c0b, c1b, c2b, c3b = (
                    s.unsqueeze(2).broadcast_to([128, nf, 4]) for s in cs)
                if on_gpsimd:
                    tt = p.tile([128, nf * 4], BF16, tag=f"tt{l}{pi}")
                    tv = tt[:].rearrange("p (m q) -> p m q", q=4)
                    vv = p.tile([128, nf * 4], BF16, tag=f"vv{l}{pi}")
                    vvv = vv[:].rearrange("p (m q) -> p m q", q=4)
                    nc.gpsimd.tensor_tensor(tv, A, Bv, op=MUL)
                    nc.gpsimd.tensor_tensor(vvv, tv, c3b, op=MUL)
                    u1 = p.tile([128, nf * 4], BF16, tag=f"u1{l}{pi}")
                    u1v = u1[:].rearrange("p (m q) -> p m q", q=4)
                    u2 = p.tile([128, nf * 4], BF16, tag=f"u2{l}{pi}")
                    u2v = u2[:].rearrange("p (m q) -> p m q", q=4)
                    nc.vector.tensor_tensor(u1v, A, c1b, op=MUL)
                    nc.vector.tensor_tensor(u2v, Bv, c2b, op=MUL)
                    nc.vector.tensor_tensor(u1v, u1v, u2v, op=ADD)
                    nc.vector.tensor_tensor(u1v, u1v, c0b, op=ADD)
                    nc.vector.tensor_tensor(Hv, u1v, vvv, op=ADD)
                    return
                m1 = p.tile([128, nf * 4], BF16, tag=f"m1{l}{pi}")
                m1v = m1[:].rearrange("p (m q) -> p m q", q=4)
                m4 = p.tile([128, nf * 4], BF16, tag=f"m4{l}{pi}")
                m4v = m4[:].rearrange("p (m q) -> p m q", q=4)
                nc.vector.tensor_tensor(m1v, Bv, c3b, op=MUL)
                nc.vector.tensor_tensor(m1v, m1v, c1b, op=ADD)
                nc.vector.tensor_tensor(m1v, m1v, A, op=MUL)
                nc.vector.tensor_tensor(m4v, Bv, c2b, op=MUL)
                nc.vector.tensor_tensor(m4v, m4v, c0b, op=ADD)
                nc.vector.tensor_tensor(Hv, m1v, m4v, op=ADD)

            Hprev = None
            for l in range(L):
                nf = NS[l] * J
                H = p.tile([128, nf * 4], BF16, tag=f"H{l}")
                Hv = H[:].rearrange("p (m q) -> p m q", q=4)
                if l == 0:
                    A = a0t[:].rearrange("p (m q) -> p m q", q=4)
                    Bv = b0t[:].rearrange("p (m q) -> p m q", q=4)
                else:
                    Hp = Hprev[:].rearrange("p (m q) -> p m q", q=4)
                    A = Hp[:, 0:nf, :]
                    Bv = Hp[:, nf:2 * nf, :]
                pieces = c_slices(l)
                o = 0
                for pi, (nsp, cs) in enumerate(pieces):
                    w = nsp * J
                    eval_piece(l, pi, A[:, o:o + w, :], Bv[:, o:o + w, :],
                               cs, Hv[:, o:o + w, :],
                               on_gpsimd=(l < 2))
                    o += w
                Hprev = H

            # ---- partial GroupSum table -> blend coefficients ----
            # per-partition partial table, converted to multilinear basis
            # BEFORE the broadcast matmul (the basis change is linear)
            H5 = Hprev[:].rearrange("p (j q) -> p j q", q=4)   # [128, J, 4]
            Hred = p.tile([128, 4], F32)
            nc.vector.tensor_reduce(Hred[:], H5.transpose([0, 2, 1]), axis=X, op=ADD)
            gp = p.tile([128, 4], F32)
            up = p.tile([128, 1], F32)
            nc.vector.tensor_copy(gp[:, 0:1], Hred[:, 0:1])
            nc.vector.tensor_tensor(gp[:, 1:2], Hred[:, 1:2], Hred[:, 0:1], op=SUB)
            nc.vector.tensor_tensor(gp[:, 2:3], Hred[:, 2:3], Hred[:, 0:1], op=SUB)
            nc.vector.tensor_tensor(up[:], Hred[:, 3:4], Hred[:, 1:2], op=SUB)
            nc.vector.tensor_tensor(gp[:, 3:4], up[:], gp[:, 2:3], op=SUB)
            ps1 = psp.tile([128, 4], F32)
            nc.tensor.matmul(ps1[:], ones_m[:], gp[:], start=True, stop=True)
            g = p.tile([128, 4], F32)
            nc.scalar.copy(g[:], ps1[:])

            # ---- per-row blend of the full batch ----
            ev = p.tile([128, BROW], F32)
            nc.vector.tensor_scalar(ev[:], s0[:], g[:, 1:2], g[:, 0:1],
                                    op0=MUL, op1=ADD)
            z1 = p.tile([128, BROW], F32)
            nc.vector.scalar_tensor_tensor(z1[:], s1[:], g[:, 2:3], ev[:],
                                           op0=MUL, op1=ADD)
            z2 = p.tile([128, BROW], F32)
            nc.vector.scalar_tensor_tensor(z2[:], t01[:], g[:, 3:4], z1[:],
                                           op0=MUL, op1=ADD)

            osb = p.tile([128, BROW, 2], F32)
            nc.vector.tensor_scalar(osb[:, :, 0], z2[:], ct[:, 0:1], None, op0=MUL)
            nc.vector.tensor_scalar(osb[:, :, 1], z2[:], ct[:, 1:2], None, op0=MUL)
            nc.sync.dma_start(out.rearrange("(p a) c -> p a c", p=128), osb[:])

    nc.compile()
    return nc


def _host_blobs(x, w0, ws, idx0, idxs):
    """Compose the stream tree and build per-core input blobs."""
    x = np.asarray(x, np.float32)
    Wl = [np.asarray(w0, np.float32)] + [np.asarray(ws[i], np.float32)
                                         for i in range(L - 1)]
    Il = [np.asarray(idx0, np.int64)] + [np.asarray(idxs[i], np.int64)
                                         for i in range(L - 1)]

    S = [None] * L
    S[L - 1] = [np.arange(K)]
    for l in range(L - 1, 0, -1):
        S[l - 1] = [Il[l][0][P] for P in S[l]] + [Il[l][1][P] for P in S[l]]

    # wall: [cores, 128, (l,s), J, 16]
    wall = np.empty((N_CORES, 128, NSTOT, J, 16), np.float32)
    for l in range(L):
        for s in range(NS[l]):
            pw = Wl[l][S[l][s]]                       # [K, 16]
            pw = pw.reshape(N_CORES, J, 128, 16)      # core, j, p, i
            wall[:, :, FO[l] + s, :, :] = pw.transpose(0, 2, 1, 3)

    # layer-0 pattern inputs: a0[core, p, s, j, q] = (q >> m0) & 1
    q = np.arange(4)
    a0 = np.empty((N_CORES, 128, NS[0], J, 4), np.float32)
    b0 = np.empty((N_CORES, 128, NS[0], J, 4), np.float32)
    for s in range(NS[0]):
        m0 = Il[0][0][S[0][s]].reshape(N_CORES, J, 128)  # core, j, p
        m1 = Il[0][1][S[0][s]].reshape(N_CORES, J, 128)
        a0[:, :, s, :, :] = ((q[None, None, None, :] >> m0.transpose(0, 2, 1)[..., None]) & 1)
        b0[:, :, s, :, :] = ((q[None, None, None, :] >> m1.transpose(0, 2, 1)[..., None]) & 1)
    a0 = a0.reshape(N_CORES, 128, NS[0] * J * 4)
    b0 = b0.reshape(N_CORES, 128, NS[0] * J * 4)

    xin = np.ascontiguousarray(x.reshape(128, BROW, 2))
    in_maps = []
    for ci in range(N_CORES):
        cls = np.array([1.0, 0.0] if ci < N_CORES // 2 else [0.0, 1.0], np.float32)
        m = {
            "a0in": np.ascontiguousarray(a0[ci]).astype(ml_dtypes.bfloat16),
            "b0in": np.ascontiguousarray(b0[ci]).astype(ml_dtypes.bfloat16),
            "xin": xin,
            "clsg": np.tile(cls, (128, 1)),
        }
        for gi, (gs, gn) in enumerate(WG):
            m[f"wall{gi}"] = np.ascontiguousarray(
                wall[ci, :, gs:gs + gn, :, :].reshape(128, -1)).astype(
                    ml_dtypes.bfloat16)
        in_maps.append(m)
    return in_maps


def run(inputs, trace=False, trace_kwargs=None):
    global _compiled
    if _compiled is None:
        _compiled = _build_program()
    nc = _compiled
    in_maps = _host_blobs(inputs["x"], inputs["w0"], inputs["ws"],
                          inputs["idx0"], inputs["idxs"])
    res = run_bass_kernel_spmd(nc, in_maps, core_ids=list(range(N_CORES)),
                               trace=trace, **(trace_kwargs or {}))
    total = np.zeros((B, 2), np.float32)
    for ci in range(N_CORES):
        total += res.results[ci]["out"]
    return total, res


def kernel(x, w0, ws, idx0, idxs):
    out, _ = run({"x": x, "w0": w0, "ws": ws, "idx0": idx0, "idxs": idxs})
    return out



# revision 7
# speedup vs baseline: 1.2021x; 1.2021x over previous
"""Trainium2 Bass kernel for nn_DiffLogicPBF (difflogic network).

Algorithm
---------
The network input is binarized to 2 bits, so every batch row's entire
activation trajectory takes one of only 4 values ("patterns").  We evaluate
the network on the 4 patterns instead of 8192 rows, then blend per-row.

The per-layer gathers (connection indices) are composed on the host into a
stream tree: layer l needs its layer-(l-1) inputs in 2 permuted orders,
giving 2^(5-l) "streams" per layer (63 total), each a gather-free
elementwise evaluation.  Weights are uploaded pre-permuted per stream.

Device pipeline (v2):
  - weights arrive gate-major: W'[(u,g), (slab, p)] bf16, u = chunk mod 8.
  - ACT: exp of all weights (chunked, overlapping the DMA).
  - PE:  per 128-column slab, LoadStationary(exp-weights) x mask[128, 40]
         -> PSUM [128, 40]: the 4 multilinear coefficient sums + the
         softmax denominator per neuron instance, already in neuron-major
         layout (partition = p, free = (chunk, coeff)).
  - DVE: reciprocal_approx_fast on the denominators.
  - GpSimd: one fused op per chunk-group normalizes all 4 coefficients
         (PSUM -> SBUF bf16).
  - DVE: 6-layer multilinear eval in q-outer layout [p, 4 patterns, m]
         (all ops hit the 2x bf16 DVE mode), then the per-row blend.
Sharding: neurons split across 8 cores (512 each); each core emits its
partial GroupSum blended over the batch [128, 64]; the host adds the four
class-0 cores into logits[:,0] and the four class-1 cores into logits[:,1].
"""

from contextlib import ExitStack

import ml_dtypes
import numpy as np

import concourse.bacc as bacc
import concourse.bass as bass
import concourse.mybir as mybir
import concourse.tile as tile
from concourse.bass_utils import run_bass_kernel_spmd

F32 = mybir.dt.float32
BF16 = mybir.dt.bfloat16
ADD = mybir.AluOpType.add
SUB = mybir.AluOpType.subtract
MUL = mybir.AluOpType.mult
GT = mybir.AluOpType.is_gt
X = mybir.AxisListType.X
EXP = mybir.ActivationFunctionType.Exp

N_CORES = 8
B, K, L = 8192, 4096, 6
NS = [32, 16, 8, 4, 2, 1]          # streams per layer
J = 4                              # 512 neurons per core = 4 chunks of 128
FO = np.cumsum([0] + NS).tolist()  # stream offsets by layer
NCH = sum(NS) * J                  # 252 real chunks per core
NCHP = 256                         # padded to 32 slabs of 8 chunks
NSLAB = NCHP // 8
BROW = B // 128                    # 64 batch rows per partition

# chunk-group pipeline stages: (slab_start, slab_end) -> chunks 8*s0..8*s1
CGS = [(0, 8), (8, 16), (16, 24), (24, 32)]
# eval m-ranges (chunks) per layer: layer l covers chunks FO[l]*J .. +NS[l]*J
LOFF = [FO[l] * J for l in range(L)]

_compiled = None


def _build_program():
    nc = bacc.Bacc("TRN2", target_bir_lowering=False, debug=False,
                   num_devices=N_CORES)
    win = nc.dram_tensor("win", [128, NCHP * 16], BF16, kind="ExternalInput").ap()
    maskin = nc.dram_tensor("maskin", [128, 40], BF16, kind="ExternalInput").ap()
    a0in = nc.dram_tensor("a0in", [128, 4 * 128], BF16, kind="ExternalInput").ap()
    b0in = nc.dram_tensor("b0in", [128, 4 * 128], BF16, kind="ExternalInput").ap()
    xin = nc.dram_tensor("xin", [128, BROW, 2], F32, kind="ExternalInput").ap()
    out = nc.dram_tensor("out", [128, BROW], F32, kind="ExternalOutput").ap()

    with tile.TileContext(nc) as tc:
        with ExitStack() as ctx:
            p = ctx.enter_context(tc.tile_pool(name="p", bufs=1))
            psp = ctx.enter_context(tc.tile_pool(name="ps", bufs=1, space="PSUM"))

            # ---- input DMAs, spread across engines so they issue early ----
            # gpsimd memsets first: wb unblocks the PE warmers immediately.
            wb = p.tile([128, 64], BF16)
            nc.gpsimd.memset(wb[:], 0.0)
            maskt = p.tile([128, 40], BF16)
            nc.sync.dma_start(maskt[:], maskin[:])
            wt = p.tile([128, NCHP * 16], BF16)
            for (s0, s1) in CGS:
                nc.sync.dma_start(wt[:, s0 * 128:s1 * 128], win[:, s0 * 128:s1 * 128])
            xt = p.tile([128, BROW, 2], F32)
            nc.scalar.dma_start(xt[:], xin[:])

            # PE warmer: keep the tensor engine's activity monitor busy so
            # the real matmuls run at full clock (HAM un-throttles after
            # ~3.4us of sustained activity).
            psw = psp.tile([16, 64], F32)
            for i in range(48):
                nc.tensor.matmul(psw[:], wb[:, 0:16], wb[:],
                                 start=True, stop=True, skip_group_check=True)

            # preload the exp spline table while the weight DMA is in flight
            scr = p.tile([128, 1], F32)
            nc.scalar.activation(scr[:], wb[:, 0:1], EXP)

            a0t = p.tile([128, 4, 128], BF16)
            nc.scalar.dma_start(a0t[:], a0in[:].rearrange("p (q m) -> p q m", m=128))
            b0t = p.tile([128, 4, 128], BF16)
            nc.scalar.dma_start(b0t[:], b0in[:].rearrange("p (q m) -> p q m", m=128))

            # blend prep + constants on gpsimd/vector while weights stream in
            s0t = p.tile([128, BROW], F32)
            nc.gpsimd.tensor_scalar(s0t[:], xt[:, :, 0], 0.0, None, op0=GT)
            s1t = p.tile([128, BROW], F32)
            nc.gpsimd.tensor_scalar(s1t[:], xt[:, :, 1], 0.0, None, op0=GT)
            t01 = p.tile([128, BROW], F32)
            nc.gpsimd.tensor_tensor(t01[:], s0t[:], s1t[:], op=MUL)
            ones = p.tile([128, 128], F32)
            nc.vector.memset(ones[:], 1.0)

            # ---- exp (ACT), chunked to overlap the weight DMA ----
            E = p.tile([128, NCHP * 16], BF16)
            for (s0, s1) in CGS:
                nc.scalar.activation(E[:, s0 * 128:s1 * 128],
                                     wt[:, s0 * 128:s1 * 128], EXP)

            # ---- corner/coefficient sums on PE ----
            # slab b: stationary = E[:, 128b:128b+128] ([128 gates*subchunk,
            # 128 instance-columns]), moving = mask [128, 40] -> psum
            # [128 p, 40 = (u, t)] at column offset 64*b (bank-aligned).
            pst = psp.tile([128, NSLAB * 64], F32)
            for b in range(NSLAB):
                nc.tensor.matmul(pst[:, 64 * b:64 * b + 40],
                                 E[:, 128 * b:128 * (b + 1)], maskt[:],
                                 start=True, stop=True)

            pstap = pst[:]
            part = pstap.ap[0]

            def ps_ap(s0, s1, t):
                """[128, slabs, 8u] AP over psum coeff t (chunk-ordered)."""
                return bass.AP(tensor=pstap.tensor, offset=pstap.offset + 64 * s0 + t,
                               ap=[part, [64, s1 - s0], [5, 8]])

            def ps_ap4(s0, s1):
                """[128, 4t, slabs, 8u] AP over psum coeffs 0..3."""
                return bass.AP(tensor=pstap.tensor, offset=pstap.offset + 64 * s0,
                               ap=[part, [1, 4], [64, s1 - s0], [5, 8]])

            # ---- 1/D (DVE fast reciprocal) + fused coefficient normalize ----
            r = p.tile([128, NCHP], F32)
            Ct = p.tile([128, 4, NCHP], BF16)   # Ct[:, t, chunk]
            ctap = Ct[:]
            rap = r[:]
            for (s0, s1) in CGS:
                c0 = 8 * s0
                rout = bass.AP(tensor=rap.tensor, offset=rap.offset + c0,
                               ap=[rap.ap[0], [8, s1 - s0], [1, 8]])
                nc.vector.reciprocal_approx_fast(rout, ps_ap(s0, s1, 4))
                rb = bass.AP(tensor=rap.tensor, offset=rap.offset + c0,
                             ap=[rap.ap[0], [0, 4], [8, s1 - s0], [1, 8]])
                cslice = bass.AP(tensor=ctap.tensor, offset=ctap.offset + c0,
                                 ap=[ctap.ap[0], [NCHP, 4], [8, s1 - s0], [1, 8]])
                nc.vector.tensor_tensor(cslice, ps_ap4(s0, s1), rb, op=MUL)

            def cb(t, o, n):
                """coeff t chunks [o, o+n) broadcast over the 4 patterns."""
                return Ct[:, t, o:o + n].unsqueeze(1).broadcast_to([128, 4, n])

            # ---- evaluate the stream tree on the 4 patterns ----
            def eval_block(l, A, Bv, o, n, H):
                # v2-subchain rides on gpsimd for the two big layers so DVE
                # only runs the serial v1 chain + join there
                e2 = nc.gpsimd if l < 2 else nc.vector
                v1 = p.tile([128, 4, n], BF16, tag=f"v1_{l}_{o}")
                v2 = p.tile([128, 4, n], BF16, tag=f"v2_{l}_{o}")
                e2.tensor_tensor(v2[:], Bv, cb(2, o, n), op=MUL)
                e2.tensor_tensor(v2[:], v2[:], cb(0, o, n), op=ADD)
                nc.vector.tensor_tensor(v1[:], Bv, cb(3, o, n), op=MUL)
                nc.vector.tensor_tensor(v1[:], v1[:], cb(1, o, n), op=ADD)
                nc.vector.tensor_tensor(v1[:], v1[:], A, op=MUL)
                nc.vector.tensor_tensor(H, v1[:], v2[:], op=ADD)

            Hs = []
            for l in range(L):
                nf = NS[l] * J
                H = p.tile([128, 4, nf], BF16, tag=f"H{l}")
                if l == 0:
                    # two half-blocks so eval starts on the first chunk-group
                    eval_block(0, a0t[:, :, 0:64], b0t[:, :, 0:64],
                               LOFF[0], 64, H[:, :, 0:64])
                    eval_block(0, a0t[:, :, 64:128], b0t[:, :, 64:128],
                               LOFF[0] + 64, 64, H[:, :, 64:128])
                else:
                    Hp = Hs[l - 1]
                    eval_block(l, Hp[:, :, 0:nf], Hp[:, :, nf:2 * nf],
                               LOFF[l], nf, H[:])
                Hs.append(H)

            # ---- GroupSum partial table -> broadcast -> per-row blend ----
            Hred = p.tile([128, 4], F32)
            nc.vector.tensor_reduce(Hred[:], Hs[-1][:], axis=X, op=ADD)
            gp = p.tile([128, 4], F32)
            up = p.tile([128, 1], F32)
            nc.vector.tensor_copy(gp[:, 0:1], Hred[:, 0:1])
            nc.vector.tensor_tensor(gp[:, 1:2], Hred[:, 1:2], Hred[:, 0:1], op=SUB)
            nc.vector.tensor_tensor(gp[:, 2:3], Hred[:, 2:3], Hred[:, 0:1], op=SUB)
            nc.vector.tensor_tensor(up[:], Hred[:, 3:4], Hred[:, 1:2], op=SUB)
            nc.vector.tensor_tensor(gp[:, 3:4], up[:], gp[:, 2:3], op=SUB)
            psg = psp.tile([128, 4], F32)
            nc.tensor.matmul(psg[:], ones[:], gp[:], start=True, stop=True)
            g = p.tile([128, 4], F32)
            nc.scalar.copy(g[:], psg[:])

            ev = p.tile([128, BROW], F32)
            nc.vector.tensor_scalar(ev[:], s0t[:], g[:, 1:2], g[:, 0:1],
                                    op0=MUL, op1=ADD)
            z1 = p.tile([128, BROW], F32)
            nc.vector.scalar_tensor_tensor(z1[:], s1t[:], g[:, 2:3], ev[:],
                                           op0=MUL, op1=ADD)
            osb = p.tile([128, BROW], F32)
            nc.vector.scalar_tensor_tensor(osb[:], t01[:], g[:, 3:4], z1[:],
                                           op0=MUL, op1=ADD)
            nc.sync.dma_start(out[:], osb[:])

    nc.compile()
    return nc


def _host_blobs(x, w0, ws, idx0, idxs):
    """Compose the stream tree and build per-core input blobs."""
    x = np.asarray(x, np.float32)
    Wl = [np.asarray(w0, np.float32)] + [np.asarray(ws[i], np.float32)
                                         for i in range(L - 1)]
    Il = [np.asarray(idx0, np.int64)] + [np.asarray(idxs[i], np.int64)
                                         for i in range(L - 1)]

    S = [None] * L
    S[L - 1] = [np.arange(K)]
    for l in range(L - 1, 0, -1):
        S[l - 1] = [Il[l][0][P] for P in S[l]] + [Il[l][1][P] for P in S[l]]

    # wall[core, chunk, p, gate]; chunk (l, s, j), neuron = j*128 + p
    wall = np.zeros((N_CORES, NCHP, 128, 16), np.float32)
    m0 = np.empty((N_CORES, 128, 128), np.int64)   # [core, p, layer0-chunk]
    m1 = np.empty((N_CORES, 128, 128), np.int64)
    for l in range(L):
        for s in range(NS[l]):
            pw = Wl[l][S[l][s]]                     # [K, 16]
            pw = pw.reshape(N_CORES, J, 128, 16)    # core, j, p, g
            c0 = (FO[l] + s) * J
            wall[:, c0:c0 + J] = pw
            if l == 0:
                i0 = Il[0][0][S[0][s]].reshape(N_CORES, J, 128)
                i1 = Il[0][1][S[0][s]].reshape(N_CORES, J, 128)
                for j in range(J):
                    m0[:, :, s * J + j] = i0[:, j, :]
                    m1[:, :, s * J + j] = i1[:, j, :]

    # mask [128 = (u, g), 40 = (u', t)]
    g = np.arange(16)
    b = [(g >> i) & 1 for i in range(4)]
    coef = np.stack([b[3], b[1] - b[3], b[2] - b[3],
                     b[0] - b[1] - b[2] + b[3], np.ones(16, np.int64)], 1)
    mask = np.zeros((128, 40), np.float32)
    for u in range(8):
        mask[u * 16:(u + 1) * 16, u * 5:(u + 1) * 5] = coef

    # a0/b0 [core, p, q, layer0-chunk]
    q = np.arange(4)
    a0 = ((q[None, None, :, None] >> m0[:, :, None, :]) & 1).astype(np.float32)
    b0 = ((q[None, None, :, None] >> m1[:, :, None, :]) & 1).astype(np.float32)

    xin = np.ascontiguousarray(x.reshape(128, BROW, 2))
    in_maps = []
    for ci in range(N_CORES):
        wt = wall[ci].reshape(NSLAB, 8, 128, 16).transpose(1, 3, 0, 2)
        m = {
            "win": np.ascontiguousarray(wt.reshape(128, NCHP * 16)).astype(
                ml_dtypes.bfloat16),
            "maskin": mask.astype(ml_dtypes.bfloat16),
            "a0in": np.ascontiguousarray(a0[ci].reshape(128, 512)).astype(
                ml_dtypes.bfloat16),
            "b0in": np.ascontiguousarray(b0[ci].reshape(128, 512)).astype(
                ml_dtypes.bfloat16),
            "xin": xin,
        }
        in_maps.append(m)
    return in_maps


def run(inputs, trace=False, trace_kwargs=None):
    global _compiled
    if _compiled is None:
        _compiled = _build_program()
    nc = _compiled
    in_maps = _host_blobs(inputs["x"], inputs["w0"], inputs["ws"],
                          inputs["idx0"], inputs["idxs"])
    res = run_bass_kernel_spmd(nc, in_maps, core_ids=list(range(N_CORES)),
                               trace=trace, **(trace_kwargs or {}))
    total = np.zeros((B, 2), np.float32)
    for ci in range(N_CORES):
        total[:, 0 if ci < N_CORES // 2 else 1] += res.results[ci]["out"].reshape(B)
    return total, res


def kernel(x, w0, ws, idx0, idxs):
    out, _ = run({"x": x, "w0": w0, "ws": ws, "idx0": idx0, "idxs": idxs})
    return out


# revision 12
# speedup vs baseline: 1.2282x; 1.0217x over previous
"""Trainium2 Bass kernel for nn_DiffLogicPBF (difflogic network).

Algorithm
---------
The network input is binarized to 2 bits, so every batch row's entire
activation trajectory takes one of only 4 values ("patterns").  We evaluate
the network on the 4 patterns instead of 8192 rows, then blend per-row.

The per-layer gathers (connection indices) are composed on the host into a
stream tree: layer l needs its layer-(l-1) inputs in 2 permuted orders,
giving 2^(5-l) "streams" per layer (63 total), each a gather-free
elementwise evaluation.  Weights are uploaded pre-permuted per stream.

Device pipeline (v2):
  - weights arrive gate-major: W'[(u,g), (slab, p)] bf16, u = chunk mod 8.
  - ACT: exp of all weights (chunked, overlapping the DMA).
  - PE:  per 128-column slab, LoadStationary(exp-weights) x mask[128, 40]
         -> PSUM [128, 40]: the 4 multilinear coefficient sums + the
         softmax denominator per neuron instance, already in neuron-major
         layout (partition = p, free = (chunk, coeff)).
  - DVE: reciprocal_approx_fast on the denominators.
  - GpSimd: one fused op per chunk-group normalizes all 4 coefficients
         (PSUM -> SBUF bf16).
  - DVE: 6-layer multilinear eval in q-outer layout [p, 4 patterns, m]
         (all ops hit the 2x bf16 DVE mode), then the per-row blend.
Sharding: neurons split across 8 cores (512 each); each core emits its
partial GroupSum blended over the batch [128, 64]; the host adds the four
class-0 cores into logits[:,0] and the four class-1 cores into logits[:,1].
"""

from contextlib import ExitStack

import ml_dtypes
import numpy as np

import concourse.bacc as bacc
import concourse.bass as bass
import concourse.mybir as mybir
import concourse.tile as tile
from concourse.bass_utils import run_bass_kernel_spmd

F32 = mybir.dt.float32
BF16 = mybir.dt.bfloat16
ADD = mybir.AluOpType.add
SUB = mybir.AluOpType.subtract
MUL = mybir.AluOpType.mult
GT = mybir.AluOpType.is_gt
X = mybir.AxisListType.X
EXP = mybir.ActivationFunctionType.Exp

N_CORES = 8
B, K, L = 8192, 4096, 6
NS = [32, 16, 8, 4, 2, 1]          # streams per layer
J = 4                              # 512 neurons per core = 4 chunks of 128
FO = np.cumsum([0] + NS).tolist()  # stream offsets by layer
NCH = sum(NS) * J                  # 252 real chunks per core
NCHP = 256                         # padded to 32 slabs of 8 chunks
NSLAB = NCHP // 8
BROW = B // 128                    # 64 batch rows per partition

# chunk-group pipeline stages: (slab_start, slab_end) -> chunks 8*s0..8*s1
CGS = [(0, 8), (8, 16), (16, 24), (24, 32)]
# eval m-ranges (chunks) per layer: layer l covers chunks FO[l]*J .. +NS[l]*J
LOFF = [FO[l] * J for l in range(L)]

_compiled = None


def _build_program():
    nc = bacc.Bacc("TRN2", target_bir_lowering=False, debug=False,
                   num_devices=N_CORES)
    # win = [mask | weights]: 40 mask cols then NCHP*16 weight cols
    win = nc.dram_tensor("win", [128, 40 + NCHP * 16], BF16,
                         kind="ExternalInput").ap()
    ab0in = nc.dram_tensor("ab0in", [128, 2 * 4 * 128], BF16,
                           kind="ExternalInput").ap()
    xin = nc.dram_tensor("xin", [128, BROW, 2], F32, kind="ExternalInput").ap()
    out = nc.dram_tensor("out", [128, BROW], F32, kind="ExternalOutput").ap()

    with tile.TileContext(nc) as tc:
        with ExitStack() as ctx:
            p = ctx.enter_context(tc.tile_pool(name="p", bufs=1))
            psp = ctx.enter_context(tc.tile_pool(name="ps", bufs=1, space="PSUM"))

            # ---- input DMAs ----
            # HWDGE descriptor generation is ~30ns/desc and blocks the
            # issuing engine's sequencer, so the bulk inputs go through
            # gpsimd's SWDGE (0.34ns/desc).  Nothing DMAs from ACT.
            wb = p.tile([128, 64], BF16)
            nc.gpsimd.memset(wb[:], 0.0)
            wall = p.tile([128, 40 + NCHP * 16], BF16)
            HCOL = 40 + NCHP * 8    # split point: mask + slabs 0..15
            nc.gpsimd.dma_start(wall[:, 0:HCOL], win[:, 0:HCOL])
            nc.gpsimd.dma_start(wall[:, HCOL:], win[:, HCOL:])
            ab0t = p.tile([128, 2, 4, 128], BF16)
            nc.gpsimd.dma_start(
                ab0t[:].rearrange("p a q m -> p (a q m)"), ab0in[:])
            maskt = wall[:, 0:40]
            wt = wall[:, 40:]
            a0t, b0t = ab0t[:, 0], ab0t[:, 1]
            xt = p.tile([128, BROW, 2], F32)
            nc.sync.dma_start(xt[:], xin[:])

            # PE warmer: keep the tensor engine's activity monitor busy so
            # the real matmuls run at full clock (HAM un-throttles after
            # ~3.4us of sustained activity).
            psw = psp.tile([16, 64], F32)
            for i in range(48):
                nc.tensor.matmul(psw[:], wb[:, 0:16], wb[:],
                                 start=True, stop=True, skip_group_check=True)

            # preload the exp spline table while the weight DMA is in flight
            scr = p.tile([128, 1], F32)
            nc.scalar.activation(scr[:], wb[:, 0:1], EXP)

            ones = p.tile([128, 128], F32)
            nc.vector.memset(ones[:], 1.0)

            # ---- exp (ACT), chunked to overlap the weight DMA ----
            E = p.tile([128, NCHP * 16], BF16)
            for (s0, s1) in CGS:
                nc.scalar.activation(E[:, s0 * 128:s1 * 128],
                                     wt[:, s0 * 128:s1 * 128], EXP)

            # ---- corner/coefficient sums on PE ----
            # slab b: stationary = E[:, 128b:128b+128] ([128 gates*subchunk,
            # 128 instance-columns]), moving = mask [128, 40] -> psum
            # [128 p, 40 = (u, t)] at column offset 64*b (bank-aligned).
            pst = psp.tile([128, NSLAB * 64], F32)
            for b in range(NSLAB):
                nc.tensor.matmul(pst[:, 64 * b:64 * b + 40],
                                 E[:, 128 * b:128 * (b + 1)], maskt,
                                 start=True, stop=True)

            pstap = pst[:]
            part = pstap.ap[0]

            def ps_ap(s0, s1, t):
                """[128, slabs, 8u] AP over psum coeff t (chunk-ordered)."""
                return bass.AP(tensor=pstap.tensor, offset=pstap.offset + 64 * s0 + t,
                               ap=[part, [64, s1 - s0], [5, 8]])

            def ps_ap4(s0, s1):
                """[128, 4t, slabs, 8u] AP over psum coeffs 0..3."""
                return bass.AP(tensor=pstap.tensor, offset=pstap.offset + 64 * s0,
                               ap=[part, [1, 4], [64, s1 - s0], [5, 8]])

            # ---- 1/D (DVE fast reciprocal) + fused coefficient normalize ----
            r = p.tile([128, NCHP], F32)
            Ct = p.tile([128, 4, NCHP], BF16)   # Ct[:, t, chunk]
            ctap = Ct[:]
            rap = r[:]
            for (s0, s1) in CGS:
                c0 = 8 * s0
                rout = bass.AP(tensor=rap.tensor, offset=rap.offset + c0,
                               ap=[rap.ap[0], [8, s1 - s0], [1, 8]])
                nc.vector.reciprocal_approx_fast(rout, ps_ap(s0, s1, 4))
                rb = bass.AP(tensor=rap.tensor, offset=rap.offset + c0,
                             ap=[rap.ap[0], [0, 4], [8, s1 - s0], [1, 8]])
                cslice = bass.AP(tensor=ctap.tensor, offset=ctap.offset + c0,
                                 ap=[ctap.ap[0], [NCHP, 4], [8, s1 - s0], [1, 8]])
                nc.vector.tensor_tensor(cslice, ps_ap4(s0, s1), rb, op=MUL)

            def cb(t, o, n):
                """coeff t chunks [o, o+n) broadcast over the 4 patterns."""
                return Ct[:, t, o:o + n].unsqueeze(1).broadcast_to([128, 4, n])

            # ---- evaluate the stream tree on the 4 patterns ----
            def eval_block(l, A, Bv, o, n, H):
                # v2-subchain rides on gpsimd for the two big layers so DVE
                # only runs the serial v1 chain + join there
                v1 = p.tile([128, 4, n], BF16, tag=f"v1_{l}_{o}")
                v2 = p.tile([128, 4, n], BF16, tag=f"v2_{l}_{o}")
                nc.gpsimd.tensor_tensor(v2[:], Bv, cb(2, o, n), op=MUL)
                nc.gpsimd.tensor_tensor(v2[:], v2[:], cb(0, o, n), op=ADD)
                nc.vector.tensor_tensor(v1[:], Bv, cb(3, o, n), op=MUL)
                nc.vector.tensor_tensor(v1[:], v1[:], cb(1, o, n), op=ADD)
                nc.vector.tensor_tensor(v1[:], v1[:], A, op=MUL)
                nc.vector.tensor_tensor(H, v1[:], v2[:], op=ADD)

            def eval_block_fused(l, A, Bv, o, n, H):
                # small layers: fuse the v1/v2 subchains into double-width
                # ops (fewer DVE instructions -> fewer inter-op bubbles).
                # w[:, 0] = C2*B + C0 (v2);  w[:, 1] = C3*B + C1 (v1 head)
                w = p.tile([128, 2, 4, n], BF16, tag=f"w_{l}_{o}")
                Bb = Bv.unsqueeze(1).broadcast_to([128, 2, 4, n])
                CA = Ct[:, 2:4, o:o + n].unsqueeze(2).broadcast_to([128, 2, 4, n])
                DA = Ct[:, 0:2, o:o + n].unsqueeze(2).broadcast_to([128, 2, 4, n])
                v1 = p.tile([128, 4, n], BF16, tag=f"v1_{l}_{o}")
                nc.vector.tensor_tensor(w[:], Bb, CA, op=MUL)
                nc.vector.tensor_tensor(w[:], w[:], DA, op=ADD)
                nc.vector.tensor_tensor(v1[:], w[:, 1], A, op=MUL)
                nc.vector.tensor_tensor(H, v1[:], w[:, 0], op=ADD)

            Hs = []
            for l in range(L):
                nf = NS[l] * J
                H = p.tile([128, 4, nf], BF16, tag=f"H{l}")
                if l == 0:
                    # two half-blocks so eval starts on the first chunk-group
                    eval_block(0, a0t[:, :, 0:64], b0t[:, :, 0:64],
                               LOFF[0], 64, H[:, :, 0:64])
                    eval_block(0, a0t[:, :, 64:128], b0t[:, :, 64:128],
                               LOFF[0] + 64, 64, H[:, :, 64:128])
                else:
                    Hp = Hs[l - 1]
                    blk = eval_block if l < 2 else eval_block_fused
                    blk(l, Hp[:, :, 0:nf], Hp[:, :, nf:2 * nf],
                        LOFF[l], nf, H[:])
                Hs.append(H)

            # blend prep on gpsimd (x lands well before the eval finishes)
            s0t = p.tile([128, BROW], F32)
            nc.gpsimd.tensor_scalar(s0t[:], xt[:, :, 0], 0.0, None, op0=GT)
            s1t = p.tile([128, BROW], F32)
            nc.gpsimd.tensor_scalar(s1t[:], xt[:, :, 1], 0.0, None, op0=GT)
            t01 = p.tile([128, BROW], F32)
            nc.gpsimd.tensor_tensor(t01[:], s0t[:], s1t[:], op=MUL)

            # ---- GroupSum partial table -> broadcast -> per-row blend ----
            Hred = p.tile([128, 4], F32)
            nc.vector.tensor_reduce(Hred[:], Hs[-1][:], axis=X, op=ADD)
            gp = p.tile([128, 4], F32)
            up = p.tile([128, 1], F32)
            nc.vector.tensor_copy(gp[:, 0:1], Hred[:, 0:1])
            nc.vector.tensor_tensor(gp[:, 1:2], Hred[:, 1:2], Hred[:, 0:1], op=SUB)
            nc.vector.tensor_tensor(gp[:, 2:3], Hred[:, 2:3], Hred[:, 0:1], op=SUB)
            nc.vector.tensor_tensor(up[:], Hred[:, 3:4], Hred[:, 1:2], op=SUB)
            nc.vector.tensor_tensor(gp[:, 3:4], up[:], gp[:, 2:3], op=SUB)
            psg = psp.tile([128, 4], F32)
            nc.tensor.matmul(psg[:], ones[:], gp[:], start=True, stop=True)
            g = p.tile([128, 4], F32)
            nc.vector.tensor_copy(g[:], psg[:])

            ev = p.tile([128, BROW], F32)
            nc.vector.tensor_scalar(ev[:], s0t[:], g[:, 1:2], g[:, 0:1],
                                    op0=MUL, op1=ADD)
            z1 = p.tile([128, BROW], F32)
            nc.vector.scalar_tensor_tensor(z1[:], s1t[:], g[:, 2:3], ev[:],
                                           op0=MUL, op1=ADD)
            osb = p.tile([128, BROW], F32)
            nc.vector.scalar_tensor_tensor(osb[:], t01[:], g[:, 3:4], z1[:],
                                           op0=MUL, op1=ADD)
            nc.gpsimd.dma_start(out[:], osb[:])

    nc.compile()
    return nc


def _host_blobs(x, w0, ws, idx0, idxs):
    """Compose the stream tree and build per-core input blobs."""
    x = np.asarray(x, np.float32)
    Wl = [np.asarray(w0, np.float32)] + [np.asarray(ws[i], np.float32)
                                         for i in range(L - 1)]
    Il = [np.asarray(idx0, np.int64)] + [np.asarray(idxs[i], np.int64)
                                         for i in range(L - 1)]

    S = [None] * L
    S[L - 1] = [np.arange(K)]
    for l in range(L - 1, 0, -1):
        S[l - 1] = [Il[l][0][P] for P in S[l]] + [Il[l][1][P] for P in S[l]]

    # wall[core, chunk, p, gate]; chunk (l, s, j), neuron = j*128 + p
    wall = np.zeros((N_CORES, NCHP, 128, 16), np.float32)
    m0 = np.empty((N_CORES, 128, 128), np.int64)   # [core, p, layer0-chunk]
    m1 = np.empty((N_CORES, 128, 128), np.int64)
    for l in range(L):
        for s in range(NS[l]):
            pw = Wl[l][S[l][s]]                     # [K, 16]
            pw = pw.reshape(N_CORES, J, 128, 16)    # core, j, p, g
            c0 = (FO[l] + s) * J
            wall[:, c0:c0 + J] = pw
            if l == 0:
                i0 = Il[0][0][S[0][s]].reshape(N_CORES, J, 128)
                i1 = Il[0][1][S[0][s]].reshape(N_CORES, J, 128)
                for j in range(J):
                    m0[:, :, s * J + j] = i0[:, j, :]
                    m1[:, :, s * J + j] = i1[:, j, :]

    # mask [128 = (u, g), 40 = (u', t)]
    g = np.arange(16)
    b = [(g >> i) & 1 for i in range(4)]
    coef = np.stack([b[3], b[1] - b[3], b[2] - b[3],
                     b[0] - b[1] - b[2] + b[3], np.ones(16, np.int64)], 1)
    mask = np.zeros((128, 40), np.float32)
    for u in range(8):
        mask[u * 16:(u + 1) * 16, u * 5:(u + 1) * 5] = coef

    # a0/b0 [core, p, q, layer0-chunk]
    q = np.arange(4)
    a0 = ((q[None, None, :, None] >> m0[:, :, None, :]) & 1).astype(np.float32)
    b0 = ((q[None, None, :, None] >> m1[:, :, None, :]) & 1).astype(np.float32)

    xin = np.ascontiguousarray(x.reshape(128, BROW, 2))
    maskb = mask.astype(ml_dtypes.bfloat16)
    in_maps = []
    for ci in range(N_CORES):
        wt = wall[ci].reshape(NSLAB, 8, 128, 16).transpose(1, 3, 0, 2)
        winb = np.concatenate(
            [maskb, wt.reshape(128, NCHP * 16).astype(ml_dtypes.bfloat16)], 1)
        ab = np.concatenate(
            [a0[ci].reshape(128, 512), b0[ci].reshape(128, 512)], 1)
        m = {
            "win": np.ascontiguousarray(winb),
            "ab0in": np.ascontiguousarray(ab).astype(ml_dtypes.bfloat16),
            "xin": xin,
        }
        in_maps.append(m)
    return in_maps


def run(inputs, trace=False, trace_kwargs=None):
    global _compiled
    if _compiled is None:
        _compiled = _build_program()
    nc = _compiled
    in_maps = _host_blobs(inputs["x"], inputs["w0"], inputs["ws"],
                          inputs["idx0"], inputs["idxs"])
    res = run_bass_kernel_spmd(nc, in_maps, core_ids=list(range(N_CORES)),
                               trace=trace, **(trace_kwargs or {}))
    total = np.zeros((B, 2), np.float32)
    for ci in range(N_CORES):
        total[:, 0 if ci < N_CORES // 2 else 1] += res.results[ci]["out"].reshape(B)
    return total, res


def kernel(x, w0, ws, idx0, idxs):
    out, _ = run({"x": x, "w0": w0, "ws": ws, "idx0": idx0, "idxs": idxs})
    return out
